# revision 21
# baseline (speedup 1.0000x reference)
"""ChunkLayer (segment-mean over boundary-delimited chunks) on 8 trn2 cores.

Full-input contract: kernel(x[8,4096,1024] f32, boundaries[8,4096] f32)
-> (means[8,128,1024] f32, cnts[8,128] i32).

Sharding: pure data parallel over batch; core i processes sequence i.

Per-core algorithm:
  - seg ids via thresholded boundaries -> hw prefix scan (free dim) +
    triangular matmul (partition-dim exclusive cumsum of row totals).
  - segment sums via one-hot matmul on the TensorEngine, accumulated in
    PSUM across 32 token tiles; counts via a ones-column matmul.
  - fp32 x is split x = hi + lo (hi = x rounded to f32r/TF32 on ScalarE,
    lo = x - hi on VectorE); both halves matmul at 1 cycle/row in f32r
    and accumulate into the same PSUM banks, recovering ~fp32 precision
    at 2x the fp32-matmul throughput. The last two tiles take the plain
    fp32-matmul path so the tail skips the ACT->DVE chain.
  - means = sums * (cnt>0)/max(cnt,1), fused into the PSUM->SBUF copy.
"""

import os

import numpy as np

import concourse.bass as bass
import concourse.tile as tile
from concourse import bacc, mybir
from concourse.bass_utils import run_bass_kernel_spmd
from concourse.masks import make_upper_triangular

B, L, D = 8, 4096, 1024
C = 128            # MAX_CHUNKS
P = 128            # SBUF partitions
T = L // P         # 32 tokens per partition; token l = p*T + t
N_CORES = 8
HALF = D // 2      # 512 = one PSUM bank of fp32

# DMA group sizes (token-tiles per dma_start). Staircase at both ends:
# small first groups let the first matmul begin early; small last groups
# shrink the post-DMA drain through the ACT->DVE->PE chain.
GROUPS = [1, 1] + [2] * 14 + [1, 1]
XPOOL_BUFS = 8
assert sum(GROUPS) == T

F32 = mybir.dt.float32
F32R = mybir.dt.float32r
I32 = mybir.dt.int32
ALU = mybir.AluOpType

# "tf32split": hi/lo f32r matmuls (fast, ~fp32 precision).
# "fp32": plain fp32 matmuls (4 cycles/row, exact).
MODE = os.environ.get("CHUNK_MODE", "tf32split")
# number of plain-fp32 tiles at the tail (only in tf32split mode)
FP32_TAIL = 2


def _emit_body(nc, pools, consts, x_r, b_r, means_out, cnts_out, tf32, mm_t):
    singles, xpool, hpool, opool, psum = pools
    iota_f32, tri_excl, ones_col, zeros_row = consts

    # ---- segment ids ----
    bnd = singles.tile([P, T], F32, name="bnd")
    nc.sync.dma_start(out=bnd, in_=b_r)

    bmask = singles.tile([P, T], F32, name="bmask")
    nc.vector.tensor_scalar(
        out=bmask, in0=bnd, scalar1=0.5, scalar2=None, op0=ALU.is_gt
    )
    cum = singles.tile([P, T], F32, name="cum")  # inclusive cumsum along t
    nc.vector.tensor_tensor_scan(
        out=cum, data0=bmask, data1=zeros_row, initial=0.0,
        op0=ALU.add, op1=ALU.add,
    )
    offs_psum = psum.tile([P, 1], F32, space="PSUM", name="offs_psum")
    nc.tensor.matmul(
        out=offs_psum, lhsT=tri_excl, rhs=cum[:, T - 1 : T],
        start=True, stop=True,
    )
    offs_m1 = singles.tile([P, 1], F32, name="offs_m1")
    nc.vector.tensor_scalar_add(out=offs_m1, in0=offs_psum, scalar1=-1.0)
    seg = singles.tile([P, T], F32, name="seg")
    nc.vector.tensor_scalar_add(out=seg, in0=cum, scalar1=offs_m1)

    # ---- main loop: segment sums via one-hot matmul ----
    sums0 = psum.tile([P, HALF], F32, space="PSUM", name="sums0")
    sums1 = psum.tile([P, HALF], F32, space="PSUM", name="sums1")
    cnt_psum = psum.tile([P, 2], F32, space="PSUM", name="cnt_psum")

    t0 = 0
    for gi, gsz in enumerate(GROUPS):
        xg = xpool.tile([P, max(GROUPS), D], F32, tag="xg", name="xg")
        nc.sync.dma_start(out=xg[:, 0:gsz, :], in_=x_r[:, t0 : t0 + gsz, :])
        for j in range(gsz):
            t = t0 + j
            first, last = t == 0, t == T - 1
            use_split = tf32 and t < T - FP32_TAIL
            onehot = opool.tile(
                [P, C], mm_t if use_split else F32,
                tag="oh_r" if use_split else "oh_f", name="onehot",
            )
            nc.vector.tensor_scalar(
                out=onehot, in0=iota_f32, scalar1=seg[:, t : t + 1],
                scalar2=None, op0=ALU.is_equal,
            )
            nc.tensor.matmul(
                out=cnt_psum, lhsT=onehot, rhs=ones_col.bitcast(onehot.dtype),
                start=first, stop=last,
            )
            if use_split:
                hi = hpool.tile([P, D], F32R, tag="hi", name="hi")
                nc.scalar.copy(hi, xg[:, j, :])
                lo = hpool.tile([P, D], F32R, tag="lo", name="lo")
                nc.vector.tensor_tensor(
                    out=lo, in0=xg[:, j, :], in1=hi.bitcast(F32),
                    op=ALU.subtract,
                )
                nc.tensor.matmul(
                    out=sums0, lhsT=onehot, rhs=hi[:, 0:HALF],
                    start=first, stop=False,
                )
                nc.tensor.matmul(
                    out=sums1, lhsT=onehot, rhs=hi[:, HALF:D],
                    start=first, stop=False,
                )
                nc.tensor.matmul(
                    out=sums0, lhsT=onehot, rhs=lo[:, 0:HALF],
                    start=False, stop=last,
                )
                nc.tensor.matmul(
                    out=sums1, lhsT=onehot, rhs=lo[:, HALF:D],
                    start=False, stop=last,
                )
            else:
                nc.tensor.matmul(
                    out=sums0, lhsT=onehot, rhs=xg[:, j, 0:HALF],
                    start=first, stop=last,
                )
                nc.tensor.matmul(
                    out=sums1, lhsT=onehot, rhs=xg[:, j, HALF:D],
                    start=first, stop=last,
                )
        t0 += gsz

    # ---- epilogue ----
    cnt_f32 = singles.tile([P, 1], F32, name="cnt_f32")
    nc.vector.tensor_copy(cnt_f32, cnt_psum[:, 0:1])
    mask = singles.tile([P, 1], F32, name="mask")
    nc.vector.tensor_scalar(
        out=mask, in0=cnt_f32, scalar1=0.0, scalar2=None, op0=ALU.is_gt
    )
    clamped = singles.tile([P, 1], F32, name="clamped")
    nc.vector.tensor_scalar_max(out=clamped, in0=cnt_f32, scalar1=1.0)
    recip = singles.tile([P, 1], F32, name="recip")
    scratch = singles.tile([P, 1], F32, name="scratch")
    nc.vector.reciprocal_approx_accurate(out=recip, in_=clamped, scratch=scratch)
    scale = singles.tile([P, 1], F32, name="scale")
    nc.vector.tensor_mul(out=scale, in0=recip, in1=mask)

    means_sb = singles.tile([P, D], F32, name="means_sb")
    nc.vector.tensor_scalar_mul(out=means_sb[:, 0:HALF], in0=sums0, scalar1=scale)
    nc.vector.tensor_scalar_mul(out=means_sb[:, HALF:D], in0=sums1, scalar1=scale)
    cnt_i32 = singles.tile([P, 1], I32, name="cnt_i32")
    nc.vector.tensor_copy(cnt_i32, cnt_f32)

    nc.sync.dma_start(out=means_out.ap(), in_=means_sb)
    nc.sync.dma_start(out=cnts_out.ap()[:, None], in_=cnt_i32)


def build_nc(repeat: int = 1) -> bass.Bass:
    nc = bacc.Bacc("TRN2", target_bir_lowering=False)

    x_in = nc.dram_tensor("x", [L, D], F32, kind="ExternalInput")
    b_in = nc.dram_tensor("boundaries", [L], F32, kind="ExternalInput")
    means_out = nc.dram_tensor("means", [C, D], F32, kind="ExternalOutput")
    cnts_out = nc.dram_tensor("cnts", [C], I32, kind="ExternalOutput")

    x_r = x_in.ap().rearrange("(p t) d -> p t d", t=T)      # [128, 32, 1024]
    b_r = b_in.ap().rearrange("(p t) -> p t", t=T)          # [128, 32]

    tf32 = MODE == "tf32split"
    mm_t = F32R if tf32 else F32

    with tile.TileContext(nc) as tc:
        with (
            tc.tile_pool(name="singles", bufs=1) as singles,
            tc.tile_pool(name="xpool", bufs=XPOOL_BUFS) as xpool,
            tc.tile_pool(name="hpool", bufs=4) as hpool,
            tc.tile_pool(name="opool", bufs=6) as opool,
            tc.tile_pool(name="psum", bufs=1, space="PSUM") as psum,
        ):
            # ---- constants (once, outside the repeat loop) ----
            iota_i32 = singles.tile([P, C], I32)
            nc.gpsimd.iota(iota_i32, pattern=[[1, C]], base=0, channel_multiplier=0)
            iota_f32 = singles.tile([P, C], F32)
            nc.vector.tensor_copy(iota_f32, iota_i32)

            tri_excl = singles.tile([P, P], F32)  # tri[k,m]=1 iff k<m
            make_upper_triangular(nc, tri_excl, val=1.0, diag=False)

            # 2 columns: f32r matmuls require an even moving free size
            ones_f32 = singles.tile([P, 2], F32)
            nc.vector.memset(ones_f32, 1.0)
            if tf32:
                ones_col = singles.tile([P, 2], F32R)
                nc.vector.tensor_copy(ones_col, ones_f32)
            else:
                ones_col = ones_f32
            zeros_row = singles.tile([P, T], F32)
            nc.vector.memset(zeros_row, 0.0)

            pools = (singles, xpool, hpool, opool, psum)
            consts = (iota_f32, tri_excl, ones_col, zeros_row)
            args = (nc, pools, consts, x_r, b_r, means_out, cnts_out, tf32, mm_t)

            if repeat == 1:
                _emit_body(*args)
            else:
                with tc.For_i(0, repeat, 1):
                    _emit_body(*args)

    nc.finalize()
    return nc


_NC_CACHE: bass.Bass | None = None


def _get_nc() -> bass.Bass:
    global _NC_CACHE
    if _NC_CACHE is None:
        _NC_CACHE = build_nc()
    return _NC_CACHE


def run(x: np.ndarray, boundaries: np.ndarray, nc=None, **spmd_kwargs):
    """Shard along batch, run on 8 cores, gather. Returns (means, cnts, res)."""
    if nc is None:
        nc = _get_nc()
    in_maps = [
        {
            "x": np.ascontiguousarray(x[i], dtype=np.float32),
            "boundaries": np.ascontiguousarray(boundaries[i], dtype=np.float32),
        }
        for i in range(N_CORES)
    ]
    res = run_bass_kernel_spmd(nc, in_maps, core_ids=list(range(N_CORES)), **spmd_kwargs)
    means = np.stack([r["means"] for r in res.results])
    cnts = np.stack([r["cnts"] for r in res.results]).astype(np.int32)
    return means, cnts, res


def kernel(x: np.ndarray, boundaries: np.ndarray):
    means, cnts, _ = run(x, boundaries)
    return means, cnts
